# revision 2
# baseline (speedup 1.0000x reference)
"""Blinn-Phong environment-map shader on 8 Trainium2 NeuronCores (v3).

Sharding: data-parallel over image rows H; core i shades rows [64*i, 64*(i+1)).

Host preps normalized n-hat / v-hat / n*v per pixel and ships them as fp16
hi/lo row pairs in one packed BIG tensor [128, 16384] per core (2 strips of
16384 pixels).  All device matmuls are single-pass fp16 with near-fp32
effective precision (lo rows carry the fp16 residuals; weight blocks carry
the matching L hi/lo factors).  Per (pixel, light) the device computes
  b   = 2 + 2 v.L        (pad-row constant via the weight matrix)
  a   = n.v + n.L
  s^p = exp(p*(ln relu(a) - 0.5 ln b))      (Ln/Exp on ScalarE, fp16)
  wv  = relu(kd n.L) + K2 s^p
and contracts wv with the light colors on the PE.  The n-hat output and a
tiny set of near-antiparallel (pixel, light) pairs are handled on the host.

Row map per strip g (base_b = 32 if g==0 else 0; base_a = 64 if g==0 else 96):
  base_b + 0-2 vh, 3-5 vh(dup), 6-8 vl, 9 one-pad   (b matmul, 32-row group)
  base_a + 0-2 nvh, 3-5 nvl, 6-8 nh, 9-11 nh(dup), 12-14 nl  (a + NL matmuls)
The two strips use disjoint PE row groups, so their matmuls run concurrently.
"""

import numpy as np

H, W = 512, 512
NCORES = 8
ROWS_PER_CORE = H // NCORES          # 64
PIX = ROWS_PER_CORE * W              # 32768 pixels per core
S = 2                                # strips per core
LSTRIP = PIX // S                    # 16384 pixels per strip
T = 512                              # free-dim chunk (one PSUM bank of fp32)
NCHUNK = LSTRIP // T                 # 32 chunks
NLIGHT = 128
EPS = 1e-6
B0 = 6e-3      # host pair-patch threshold on b = 2+2 v.L
B_PIX = 1e-4   # host full-reshade threshold on min_k b


def _f16r(x):
    return x.astype(np.float16).astype(np.float32)


def _hilo(x):
    h = x.astype(np.float16)
    l = (x - h.astype(np.float32)).astype(np.float16)
    return h, l


def _base_b(g):
    return 32 if g == 0 else 0


def _base_a(g):
    return 64 if g == 0 else 96


def _pack_big(n_ex, v_ex, nv_ex):
    """[PIX,3] fp32 each -> [128, LSTRIP] fp16 packed rows."""
    big = np.zeros((128, LSTRIP), np.float16)
    for g in range(S):
        sl = slice(g * LSTRIP, (g + 1) * LSTRIP)
        bb, ba = _base_b(g), _base_a(g)
        vh, vl = _hilo(v_ex[sl])
        nh, nl = _hilo(n_ex[sl])
        nvh, nvl = _hilo(nv_ex[sl])
        big[bb + 0:bb + 3] = vh.T
        big[bb + 3:bb + 6] = vh.T
        big[bb + 6:bb + 9] = vl.T
        big[bb + 9] = 1.0
        big[ba + 0:ba + 3] = nvh.T
        big[ba + 3:ba + 6] = nvl.T
        big[ba + 6:ba + 9] = nh.T
        big[ba + 9:ba + 12] = nh.T
        big[ba + 12:ba + 15] = nl.T
    return np.ascontiguousarray(big)


def _build_weights(light_directions, light_colors, kd):
    L = np.asarray(light_directions, np.float32)
    C = np.asarray(light_colors, np.float32)
    Lh, Ll = _hilo(L)
    L2h, L2l = _hilo(2.0 * L)
    kLh, kLl = _hilo(kd * L)

    w3 = np.zeros((128, 6 * NLIGHT), np.float16)
    for g in range(S):
        bb, ba = _base_b(g), _base_a(g)
        cb = g * NLIGHT                 # b-matmul block
        ca = (2 + g) * NLIGHT           # a-matmul block
        cn = (4 + g) * NLIGHT           # NL block
        for c in range(3):
            w3[bb + 0 + c, cb:cb + NLIGHT] = L2h[:, c]
            w3[bb + 3 + c, cb:cb + NLIGHT] = L2l[:, c]
            w3[bb + 6 + c, cb:cb + NLIGHT] = L2h[:, c]
            w3[ba + 0 + c, ca:ca + NLIGHT] = np.float16(1.0)
            w3[ba + 3 + c, ca:ca + NLIGHT] = np.float16(1.0)
            w3[ba + 6 + c, ca:ca + NLIGHT] = Lh[:, c]
            w3[ba + 9 + c, ca:ca + NLIGHT] = Ll[:, c]
            w3[ba + 12 + c, ca:ca + NLIGHT] = Lh[:, c]
            w3[ba + 6 + c, cn:cn + NLIGHT] = kLh[:, c]
            w3[ba + 9 + c, cn:cn + NLIGHT] = kLl[:, c]
        w3[bb + 9, cb:cb + NLIGHT] = np.float16(2.0)
    wc = np.ascontiguousarray(C.astype(np.float16))   # [128, 3]
    return w3, wc


def _build_program(p_imm, lnK2):
    import concourse.bacc as bacc
    import concourse.tile as tile
    import concourse.mybir as mybir
    from contextlib import ExitStack

    f32 = mybir.dt.float32
    f16 = mybir.dt.float16
    Alu = mybir.AluOpType
    Act = mybir.ActivationFunctionType

    # Only Ln/Exp are used; keep the combined table set always selected so no
    # ACT_TABLE_LOAD switches are inserted (ids are positional: keep the set
    # list/order, strip Ln/Exp from the other sets).
    if not hasattr(bacc, "_orig_get_activation_tables"):
        bacc._orig_get_activation_tables = bacc.get_activation_tables

    def _one_set(arch):
        t = bacc._orig_get_activation_tables(arch)
        ln = mybir.ActivationFunctionType.Ln
        ex = mybir.ActivationFunctionType.Exp
        out = {}
        for name, funcs in t.items():
            if name == "natural_log_exp_and_others":
                out[name] = funcs
            else:
                out[name] = funcs - {ln, ex}
        return out

    bacc.get_activation_tables = _one_set

    nc = bacc.Bacc("TRN2", target_bir_lowering=False, debug=False,
                   num_devices=NCORES)

    bigd = nc.declare_dram_parameter("big", [128, LSTRIP], f16, isOutput=False)
    w3d = nc.declare_dram_parameter("w3", [128, 6 * NLIGHT], f16, isOutput=False)
    wcd = nc.declare_dram_parameter("wc", [NLIGHT, 3], f16, isOutput=False)
    o_col = nc.declare_dram_parameter("o_col", [3 * S, LSTRIP], f16, isOutput=True)

    with tile.TileContext(nc) as tc, ExitStack() as ctx:
        cpool = ctx.enter_context(tc.tile_pool(name="const", bufs=1))
        sp = ctx.enter_context(tc.tile_pool(name="stream", bufs=6))
        pp = ctx.enter_context(tc.tile_pool(name="pair", bufs=3))
        wvp = ctx.enter_context(tc.tile_pool(name="wv", bufs=4))
        mmp = ctx.enter_context(tc.tile_pool(name="mm", bufs=2, space="PSUM"))
        mm2p = ctx.enter_context(tc.tile_pool(name="mm2", bufs=1, space="PSUM"))
        colp = ctx.enter_context(tc.tile_pool(name="colp", bufs=2, space="PSUM"))

        W3 = cpool.tile([128, 6 * NLIGHT], f16, tag="W3")
        WC = cpool.tile([NLIGHT, 3], f16, tag="WC")
        BK = cpool.tile([128, 1], f32, tag="BK")
        nc.gpsimd.dma_start(W3[:], w3d[:])
        nc.gpsimd.dma_start(WC[:], wcd[:])
        nc.vector.memset(BK[:], lnK2)

        CPS = colp.tile([128, T], f32, tag="CPS")
        for j in range(NCHUNK):
            cs = slice(j * T, (j + 1) * T)
            BIGT = sp.tile([128, T], f16, tag="BIGT")
            nc.gpsimd.dma_start(BIGT[:], bigd[:, cs])

            TA2 = pp.tile([128, 2 * T], f16, tag="TA2")
            LNB2 = pp.tile([128, 2 * T], f16, tag="LNB2")
            LNA2 = pp.tile([128, 2 * T], f16, tag="LNA2")
            T2 = pp.tile([128, 2 * T], f16, tag="T2")
            SPEC2 = pp.tile([128, 2 * T], f16, tag="SPEC2")
            BPS2 = mm2p.tile([128, 2 * T], f32, tag="BPS2")
            APS2 = mm2p.tile([128, 2 * T], f32, tag="APS2")
            NLP2 = mm2p.tile([128, 2 * T], f32, tag="NLP2")
            WV2 = wvp.tile([128, 2 * T], f16, tag="WV2")
            for g in range(S):
                bb, ba = _base_b(g), _base_a(g)
                cb, ca, cn = g * NLIGHT, (2 + g) * NLIGHT, (4 + g) * NLIGHT
                hs = slice(g * T, (g + 1) * T)
                nc.tensor.matmul(out=BPS2[:, hs], lhsT=W3[bb:bb + 32, cb:cb + NLIGHT],
                                 rhs=BIGT[bb:bb + 32, :], start=True, stop=True,
                                 tile_position=(bb, 0))
                nc.tensor.matmul(out=APS2[:, hs], lhsT=W3[ba:ba + 32, ca:ca + NLIGHT],
                                 rhs=BIGT[ba:ba + 32, :], start=True, stop=True,
                                 tile_position=(ba, 0))
                nc.tensor.matmul(out=NLP2[:, hs], lhsT=W3[ba:ba + 32, cn:cn + NLIGHT],
                                 rhs=BIGT[ba:ba + 32, :], start=True, stop=True,
                                 tile_position=(ba, 0))

            if j % 4 == 3:
                nc.scalar.activation(TA2[:], APS2[:], Act.Relu)
            else:
                nc.vector.tensor_scalar(out=TA2[:], in0=APS2[:],
                                        scalar1=0.0, scalar2=None, op0=Alu.max)
            nc.scalar.activation(LNB2[:], BPS2[:], Act.Ln)
            nc.scalar.activation(LNA2[:], TA2[:], Act.Ln)
            nc.vector.scalar_tensor_tensor(out=T2[:], in0=LNB2[:], scalar=-0.5,
                                           in1=LNA2[:], op0=Alu.mult, op1=Alu.add)
            nc.scalar.activation(SPEC2[:], T2[:], Act.Exp, bias=BK[:],
                                 scale=p_imm)
            nc.vector.scalar_tensor_tensor(out=WV2[:], in0=NLP2[:], scalar=0.0,
                                           in1=SPEC2[:], op0=Alu.max, op1=Alu.add)

            for g in range(S):
                hs = slice(g * T, (g + 1) * T)
                WV = WV2[:, hs]
                v = 2 * j + g
                q = v % 4
                nc.tensor.matmul(out=CPS[32 * q:32 * q + 3, :], lhsT=WC[:],
                                 rhs=WV, start=True, stop=True,
                                 tile_position=(0, 32 * q))
                if q == 3:
                    COLS = wvp.tile([128, T], f16, tag="COLS")
                    nc.vector.tensor_copy(COLS[:], CPS[:])
                    for i in range(4):
                        vv = v - 3 + i
                        jj, gg = vv // 2, vv % 2
                        nc.sync.dma_start(
                            o_col[3 * gg:3 * gg + 3, jj * T:(jj + 1) * T],
                            COLS[32 * i:32 * i + 3, :])
                    if v != 2 * NCHUNK - 1:
                        CPS = colp.tile([128, T], f32, tag="CPS")

    nc.compile()
    return nc


def _host_patch(colors, n_ex, v_ex, nv_ex, L, C, p, K2):
    """Fix near-antiparallel (pixel, light) pairs and degenerate pixels."""
    VL = v_ex @ L.T
    b_h = 2.0 + 2.0 * VL

    bad_pix = b_h.min(axis=1) < B_PIX
    pairmask = (b_h < B0) & ~bad_pix[:, None]
    pi, ki = np.nonzero(pairmask)

    if len(pi):
        vh, vl = _hilo(v_ex)
        nh, nl = _hilo(n_ex)
        nvh, nvl = _hilo(nv_ex)
        Lh, Ll = _hilo(L)
        L2h, L2l = _hilo(2.0 * L)
        vhf = vh.astype(np.float32); vlf = vl.astype(np.float32)
        nhf = nh.astype(np.float32); nlf = nl.astype(np.float32)
        b_rep = ((vhf[pi] * (L2h.astype(np.float32)[ki]
                             + L2l.astype(np.float32)[ki])).sum(1)
                 + (vlf[pi] * L2h.astype(np.float32)[ki]).sum(1) + 2.0)
        a_rep = (nvh.astype(np.float32)[pi].sum(1)
                 + nvl.astype(np.float32)[pi].sum(1)
                 + (nhf[pi] * (Lh.astype(np.float32)[ki]
                               + Ll.astype(np.float32)[ki])).sum(1)
                 + (nlf[pi] * Lh.astype(np.float32)[ki]).sum(1))
        ta = np.maximum(a_rep, 0.0)
        with np.errstate(divide="ignore", over="ignore", invalid="ignore"):
            s_rep_p = np.where(
                (ta > 0) & (b_rep > 0),
                np.exp(p * (np.log(np.maximum(ta, 1e-45))
                            - 0.5 * np.log(np.maximum(b_rep, 1e-45)))),
                0.0)
        u = v_ex[pi].astype(np.float64) + L[ki].astype(np.float64)
        Hv = u / np.maximum(np.linalg.norm(u, axis=1, keepdims=True), EPS)
        s_ref = np.clip((n_ex[pi].astype(np.float64) * Hv).sum(1), 0.0, 1.0)
        dcon = (s_ref ** p - s_rep_p) * K2
        np.add.at(colors, pi,
                  (dcon[:, None] * C[ki].astype(np.float64)).astype(np.float32))

    return bad_pix


def _host_reshade(colors, idx, n_ex, v_ex, L, C, p, K2, kd):
    n64 = n_ex[idx].astype(np.float64)
    v64 = v_ex[idx].astype(np.float64)
    dcl = np.clip(n64 @ L.T.astype(np.float64), 0.0, 1.0)
    diffuse = dcl @ C.astype(np.float64)
    u = v64[:, None, :] + L[None, :, :].astype(np.float64)
    Hv = u / np.maximum(np.linalg.norm(u, axis=2, keepdims=True), EPS)
    s = np.clip(np.einsum("ij,ikj->ik", n64, Hv), 0.0, 1.0)
    spec = (s ** p) @ C.astype(np.float64)
    colors[idx] = (kd * diffuse + K2 * spec).astype(np.float32)


def kernel(pixel_normals, pixel_directions, camera_position, light_directions,
           light_colors, shininess, kd, ks):
    from concourse.bass_utils import run_bass_kernel_spmd

    p = float(np.asarray(shininess).reshape(-1)[0])
    kdv = float(np.asarray(kd).reshape(-1)[0])
    ksv = float(np.asarray(ks).reshape(-1)[0])
    nf = (p + 2.0) / (4.0 * (2.0 - np.exp(-p / 2.0)))
    K2 = float(nf * ksv)
    lnK2 = float(np.log(max(K2, 1e-38)))

    L = np.asarray(light_directions, np.float32)
    C = np.asarray(light_colors, np.float32)
    cam = np.asarray(camera_position, np.float32)

    pn = np.asarray(pixel_normals, np.float32).reshape(H * W, 3)
    pd = np.asarray(pixel_directions, np.float32).reshape(H * W, 3)

    n_ex = pn / np.maximum(np.linalg.norm(pn, axis=1, keepdims=True), EPS)
    v_ = cam[None, :] - pd
    v_ex = v_ / np.maximum(np.linalg.norm(v_, axis=1, keepdims=True), EPS)
    nv_ex = n_ex * v_ex

    w3, wc = _build_weights(L, C, kdv)
    nc = _build_program(p, lnK2)

    in_maps = []
    for i in range(NCORES):
        sl = slice(i * PIX, (i + 1) * PIX)
        in_maps.append({
            "big": _pack_big(n_ex[sl], v_ex[sl], nv_ex[sl]),
            "w3": w3,
            "wc": wc,
        })

    res = run_bass_kernel_spmd(nc, in_maps, list(range(NCORES)))

    colors = np.empty((H * W, 3), np.float32)
    for i in range(NCORES):
        sl = slice(i * PIX, (i + 1) * PIX)
        oc = res.results[i]["o_col"].astype(np.float32)   # [6, LSTRIP]
        colors[sl] = oc.reshape(S, 3, LSTRIP).transpose(0, 2, 1).reshape(PIX, 3)

    bad_pix = _host_patch(colors, n_ex, v_ex, nv_ex, L, C, p, K2)
    if bad_pix.any():
        _host_reshade(colors, np.nonzero(bad_pix)[0], n_ex, v_ex, L, C, p, K2, kdv)

    return colors.reshape(H, W, 3), n_ex.reshape(H, W, 3).copy()
